# revision 2
# baseline (speedup 1.0000x reference)
"""CGCNN DOS predictor on 8 trn2 NeuronCores — v2 (instruction-minimal).

This runtime feeds ~12-15k instructions/sec regardless of tile size, so the
kernel is built around few, wide instructions:
- eaP: all 5 layers' edge_attr projections precomputed in phase 0
  (one [128,640] matmul per edge tile), loaded per window per layer.
- h kept only as the core's own shard (h_own [128, 99*64]); the bfs table
  is computed shard-wise and AllGathered.
- per window: 16 src gathers + 16 afs gather-adds (indirect DMA),
  1 wide FS assemble, 1 wide one-hot, 4 wide activations, 16 scatter
  matmuls, 2 pool/stat matmuls — ~60 instructions per 2048 edges.
- BN: per-core stats+pool partials ride a tiny AllGather; apply is 3 wide
  vector ops on [128, 6336].

msg = sigmoid(F)*softplus(S) via the [-F|S] trick: e=exp(.), l=ln(1+e),
sigmoid(F)=exp(-l_F), msg=sigmoid*l_S.
"""
import sys

sys.path.insert(0, "/opt/trn_rl_repo")

import numpy as np
import ml_dtypes

import concourse.bass as bass
import concourse.bacc as bacc
import concourse.tile as tile
from concourse import mybir
from concourse.bass_utils import run_bass_kernel_spmd

N_NODES = 100000
NUM_GRAPHS = 128
ATOM_IN = 92
FEA = 64
EDGE_DIM = 41
N_CONV = 5
HID = 256
LATENT = 128
CH = 3
BN_EPS = 1e-5

P = 128
NCORES = 8
NW = 99                 # windows (dst node chunks) per core
TPW = 16                # edge tiles per window
VC = NW * P             # 12672 node slots per core
VTOT = NCORES * VC      # 101376
EC = NW * TPW * P       # 202752 edge slots per core
NT = NW * TPW           # 1584 edge tiles per core
WID = TPW * P           # 2048 edges per window
PC5 = N_CONV * 128      # 640

f32 = mybir.dt.float32
bf16 = mybir.dt.bfloat16
i32 = mybir.dt.int32
AF = mybir.ActivationFunctionType
OP = mybir.AluOpType

_cache = {}


def _build():
    import os
    NL = int(os.environ.get("V2_NLAYERS", str(N_CONV)))
    DO_A = os.environ.get("V2_DO_A", "1") == "1"
    DO_B = os.environ.get("V2_DO_B", "1") == "1"
    DO_ST = os.environ.get("V2_DO_ST", "1") == "1"
    DO_EAP = os.environ.get("V2_DO_EAP", "1") == "1"
    nc = bacc.Bacc("TRN2", target_bir_lowering=False, debug=False,
                   num_devices=NCORES)

    ext_in = lambda n, s, d: nc.dram_tensor(n, s, d, kind="ExternalInput")
    h0_in = ext_in("h0_in", [VC, FEA], bf16)
    srcr_in = ext_in("srcr_in", [P, NT], i32)       # col w*16+t
    dstr_in = ext_in("dstr_in", [P, NT], i32)
    slotr_in = ext_in("slotr_in", [P, NT], bf16)
    eaT_in = ext_in("eaT_in", [EDGE_DIM, EC], bf16)
    batcho_in = ext_in("batcho_in", [P, NW], bf16)  # own nodes' graph ids
    iota_in = ext_in("iota_in", [P, P], bf16)
    Wi_in = ext_in("Wi_in", [P, N_CONV * 128], bf16)
    Wj_in = ext_in("Wj_in", [P, N_CONV * 128], bf16)
    Wea_in = ext_in("Wea_in", [EDGE_DIM, PC5], bf16)
    bias_in = ext_in("bias_in", [P, N_CONV * 128], bf16)
    gam_in = ext_in("gam_in", [1, N_CONV * FEA], f32)
    bet_in = ext_in("bet_in", [1, N_CONV * FEA], f32)
    hpool_in = ext_in("hpool_in", [P, FEA], f32)    # host pool-sum of h0
    cnt_in = ext_in("cnt_in", [P, 1], f32)          # nodes per graph
    W1_in = ext_in("W1_in", [FEA, HID], bf16)
    W2_in = ext_in("W2_in", [HID, HID], bf16)
    W3_in = ext_in("W3_in", [HID, LATENT * CH], bf16)
    b1_in = ext_in("b1_in", [P, HID], f32)
    b2_in = ext_in("b2_in", [P, HID], f32)
    b3_in = ext_in("b3_in", [P, LATENT * CH], f32)

    dos_out = nc.dram_tensor("dos_out", [P, LATENT * CH], f32,
                             kind="ExternalOutput")
    hdbg_out = nc.dram_tensor("hdbg_out", [P, FEA], f32, kind="ExternalOutput")

    eaP_dram = nc.dram_tensor("eaP_dram", [EC, PC5], bf16)
    bfs_bounce = nc.dram_tensor("bfs_bounce", [VC, P], bf16)
    bfs_all = nc.dram_tensor("bfs_all", [VTOT, P], bf16, addr_space="Shared")
    afs_dram = nc.dram_tensor("afs_dram", [VC, P], bf16)
    st_bounce = nc.dram_tensor("st_bounce", [P + 1, P], f32)
    st_all = nc.dram_tensor("st_all", [NCORES * (P + 1), P], f32,
                            addr_space="Shared")
    rg = [list(range(NCORES))]

    with tile.TileContext(nc) as tc:
        with (
            tc.tile_pool(name="const", bufs=1) as cp,
            tc.tile_pool(name="work", bufs=2) as wp,
            tc.tile_pool(name="gath", bufs=3) as gp,
            tc.tile_pool(name="small", bufs=4) as sp,
            tc.tile_pool(name="pmm", bufs=2, space="PSUM") as pmm,   # 2bk x2
            tc.tile_pool(name="ptp", bufs=1, space="PSUM") as ptp,   # 1bk
            tc.tile_pool(name="paux", bufs=1, space="PSUM") as paux,  # 1bk
            tc.tile_pool(name="pagg", bufs=1, space="PSUM") as pagg,  # 1bk
            tc.tile_pool(name="ppool", bufs=1, space="PSUM") as ppool,  # 1bk
        ):
            # ---------------- persistent SBUF ----------------
            h_own = cp.tile([P, NW * FEA], bf16, tag="h_own")
            agg_own = cp.tile([P, NW * FEA], bf16, tag="agg_own")
            bnt = cp.tile([P, NW * FEA], bf16, tag="bnt")
            B_all = cp.tile([P, NW * P], bf16, tag="B_all")
            srcr = cp.tile([P, NT], i32, tag="srcr")
            dstr = cp.tile([P, NT], i32, tag="dstr")
            slotr = cp.tile([P, NT], bf16, tag="slotr")
            iota = cp.tile([P, P], bf16, tag="iota")
            ident = cp.tile([P, P], bf16, tag="ident")
            identf = cp.tile([P, P], f32, tag="identf")
            batcho = cp.tile([P, NW], bf16, tag="batcho")
            Wi_sb = cp.tile([P, N_CONV * 128], bf16, tag="Wi")
            Wj_sb = cp.tile([P, N_CONV * 128], bf16, tag="Wj")
            Wea_sb = cp.tile([EDGE_DIM, PC5], bf16, tag="Wea")
            bias_sb = cp.tile([P, N_CONV * 128], bf16, tag="bias")
            gam_sb = cp.tile([1, N_CONV * FEA], f32, tag="gam")
            bet_sb = cp.tile([1, N_CONV * FEA], f32, tag="bet")
            hpool_sb = cp.tile([P, FEA], f32, tag="hpool")
            cnt_sb = cp.tile([P, 1], f32, tag="cnt")
            poolacc = cp.tile([P, FEA], f32, tag="poolacc")
            W1_sb = cp.tile([FEA, HID], bf16, tag="W1")
            W2a_sb = cp.tile([P, HID], bf16, tag="W2a")
            W2b_sb = cp.tile([P, HID], bf16, tag="W2b")
            W3a_sb = cp.tile([P, LATENT * CH], bf16, tag="W3a")
            W3b_sb = cp.tile([P, LATENT * CH], bf16, tag="W3b")
            b1_sb = cp.tile([P, HID], f32, tag="b1")
            b2_sb = cp.tile([P, HID], f32, tag="b2")
            b3_sb = cp.tile([P, LATENT * CH], f32, tag="b3")
            eps_t = cp.tile([1, 1], f32, tag="eps")
            onesf = cp.tile([P, 1], f32, tag="onesf")

            for t_, s_ in [(srcr, srcr_in), (dstr, dstr_in),
                           (slotr, slotr_in), (iota, iota_in),
                           (batcho, batcho_in), (Wi_sb, Wi_in),
                           (Wj_sb, Wj_in), (Wea_sb, Wea_in),
                           (bias_sb, bias_in), (gam_sb, gam_in),
                           (bet_sb, bet_in), (hpool_sb, hpool_in),
                           (cnt_sb, cnt_in), (W1_sb, W1_in),
                           (b1_sb, b1_in), (b2_sb, b2_in), (b3_sb, b3_in)]:
                nc.sync.dma_start(out=t_[:], in_=s_[:])
            nc.sync.dma_start(out=W2a_sb[:], in_=W2_in[0:P, :])
            nc.sync.dma_start(out=W2b_sb[:], in_=W2_in[P:HID, :])
            nc.sync.dma_start(out=W3a_sb[:], in_=W3_in[0:P, :])
            nc.sync.dma_start(out=W3b_sb[:], in_=W3_in[P:HID, :])
            nc.gpsimd.memset(eps_t[:], BN_EPS)
            nc.gpsimd.memset(onesf[:], 1.0)
            nc.gpsimd.memset(poolacc[:], 0.0)
            from concourse.masks import make_identity
            make_identity(nc, ident[:])
            make_identity(nc, identf[:])

            # h0 load: rows w*128+p -> h_own[p, w*64:(w+1)*64]
            nc.sync.dma_start(
                out=h_own[:].rearrange("p (w f) -> p w f", w=NW),
                in_=h0_in[:].rearrange("(w p) f -> p w f", w=NW))

            # B_all one-hot: B_all[p, w*128+g] = (batcho[p,w] == g)
            nc.vector.tensor_tensor(
                out=B_all[:].rearrange("p (w g) -> p w g", w=NW),
                in0=batcho[:, :, None].to_broadcast([P, NW, P]),
                in1=iota[:, None, :].to_broadcast([P, NW, P]),
                op=OP.is_equal)

            # ---------------- phase 0: eaP precompute ----------------
            for w in range(NW if DO_EAP else 0):
                eat = wp.tile([EDGE_DIM, WID], bf16, tag="eat")
                nc.sync.dma_start(out=eat[:],
                                  in_=eaT_in[:, w * WID:(w + 1) * WID])
                for t in range(TPW):
                    ep = pmm.tile([P, PC5], f32, space="PSUM", tag="mm4")
                    nc.tensor.matmul(out=ep[:, 0:512],
                                     lhsT=eat[:, t * P:(t + 1) * P],
                                     rhs=Wea_sb[:, 0:512],
                                     start=True, stop=True)
                    nc.tensor.matmul(out=ep[:, 512:PC5],
                                     lhsT=eat[:, t * P:(t + 1) * P],
                                     rhs=Wea_sb[:, 512:PC5],
                                     start=True, stop=True)
                    eb = wp.tile([P, PC5], bf16, tag="eb")
                    nc.vector.tensor_copy(out=eb[:], in_=ep[:])
                    gt = w * TPW + t
                    nc.sync.dma_start(out=eaP_dram[gt * P:(gt + 1) * P, :],
                                      in_=eb[:])

            # ---------------- conv layers ----------------
            for li in range(NL):
                lo = li * 128
                # ---- A: own-shard projections ----
                for cq in range(0, NW if DO_A else 0, 4):
                    ncq = min(4, NW - cq)
                    hT = wp.tile([FEA, 4 * P], bf16, tag="hT")
                    for j in range(ncq):
                        tp = ptp.tile([P, P], bf16, space="PSUM", tag="tp")
                        nc.tensor.transpose(
                            out=tp[0:FEA, :],
                            in_=h_own[:, (cq + j) * FEA:(cq + j + 1) * FEA],
                            identity=ident[:])
                        nc.vector.tensor_copy(
                            out=hT[:, j * P:(j + 1) * P],
                            in_=tp[0:FEA, :])
                    pbpa = pmm.tile([P, 8 * P], f32, space="PSUM", tag="mm4")
                    for j in range(ncq):
                        lhs = hT[:, j * P:(j + 1) * P]
                        nc.tensor.matmul(out=pbpa[:, j * P:(j + 1) * P],
                                         lhsT=lhs,
                                         rhs=Wj_sb[0:FEA, lo:lo + 128],
                                         start=True, stop=True)
                        nc.tensor.matmul(
                            out=pbpa[:, (4 + j) * P:(5 + j) * P],
                            lhsT=lhs,
                            rhs=Wi_sb[0:FEA, lo:lo + 128],
                            start=True, stop=True)
                    bpk = wp.tile([P, 4 * P], bf16, tag="bpk")
                    nc.vector.tensor_copy(out=bpk[:, 0:ncq * P],
                                          in_=pbpa[:, 0:ncq * P])
                    nc.sync.dma_start(
                        out=bfs_bounce[cq * P:(cq + ncq) * P, :]
                        .rearrange("(j p) c -> p j c", j=ncq),
                        in_=bpk[:, 0:ncq * P]
                        .rearrange("p (j c) -> p j c", j=ncq))
                    apk = wp.tile([P, 4 * P], bf16, tag="apk")
                    nc.vector.tensor_tensor(
                        out=apk[:, 0:ncq * P]
                        .rearrange("p (j c) -> p j c", j=ncq),
                        in0=pbpa[:, 4 * P:(4 + ncq) * P]
                        .rearrange("p (j c) -> p j c", j=ncq),
                        in1=bias_sb[:, None, lo:lo + 128]
                        .to_broadcast([P, ncq, P]),
                        op=OP.add)
                    nc.sync.dma_start(
                        out=afs_dram[cq * P:(cq + ncq) * P, :]
                        .rearrange("(j p) c -> p j c", j=ncq),
                        in_=apk[:, 0:ncq * P]
                        .rearrange("p (j c) -> p j c", j=ncq))

                if DO_A:
                    nc.gpsimd.collective_compute(
                        "AllGather", OP.bypass, replica_groups=rg,
                        ins=[bfs_bounce[:]], outs=[bfs_all[:]])

                # ---- B: edge pass ----
                poolP = ppool.tile([P, P], f32, space="PSUM", tag="poolP")
                for w in range(NW if DO_B else 0):
                    G = gp.tile([P, WID], bf16, tag="G")
                    for t in range(TPW):
                        gti = w * TPW + t
                        nc.gpsimd.indirect_dma_start(
                            out=G[:, t * P:(t + 1) * P], out_offset=None,
                            in_=bfs_all[:],
                            in_offset=bass.IndirectOffsetOnAxis(
                                ap=srcr[:, gti:gti + 1], axis=0))
                    for t in range(TPW):
                        gti = w * TPW + t
                        nc.gpsimd.indirect_dma_start(
                            out=G[:, t * P:(t + 1) * P], out_offset=None,
                            in_=afs_dram[:],
                            in_offset=bass.IndirectOffsetOnAxis(
                                ap=dstr[:, gti:gti + 1], axis=0),
                            compute_op=OP.add)
                    eaPw = gp.tile([P, WID], bf16, tag="eaPw")
                    nc.sync.dma_start(
                        out=eaPw[:].rearrange("p (t c) -> p t c", t=TPW),
                        in_=eaP_dram[w * WID:(w + 1) * WID, lo:lo + 128]
                        .rearrange("(t p) c -> p t c", t=TPW))
                    FS = wp.tile([P, WID], bf16, tag="FS")
                    nc.vector.tensor_tensor(out=FS[:], in0=G[:], in1=eaPw[:],
                                            op=OP.add)
                    oh = wp.tile([P, WID], bf16, tag="oh")
                    nc.vector.tensor_tensor(
                        out=oh[:].rearrange("p (t s) -> p t s", t=TPW),
                        in0=slotr[:, w * TPW:(w + 1) * TPW, None]
                        .to_broadcast([P, TPW, P]),
                        in1=iota[:, None, :].to_broadcast([P, TPW, P]),
                        op=OP.is_equal)
                    e1 = wp.tile([P, WID], bf16, tag="e1")
                    nc.scalar.activation(out=e1[:], in_=FS[:], func=AF.Exp)
                    l1 = wp.tile([P, WID], bf16, tag="l1")
                    nc.scalar.activation(out=l1[:], in_=e1[:], func=AF.Ln,
                                         bias=1.0)
                    sg = wp.tile([P, TPW * FEA], bf16, tag="sg")
                    nc.scalar.activation(
                        out=sg[:].rearrange("p (t f) -> p t f", t=TPW),
                        in_=l1[:].rearrange("p (t s) -> p t s", t=TPW)
                        [:, :, 0:FEA],
                        func=AF.Exp, scale=-1.0)
                    msg = wp.tile([P, TPW * FEA], bf16, tag="msg")
                    nc.vector.tensor_tensor(
                        out=msg[:].rearrange("p (t f) -> p t f", t=TPW),
                        in0=sg[:].rearrange("p (t f) -> p t f", t=TPW),
                        in1=l1[:].rearrange("p (t s) -> p t s", t=TPW)
                        [:, :, FEA:P],
                        op=OP.mult)
                    aggP = pagg.tile([P, FEA], f32, space="PSUM", tag="aggP")
                    for t in range(TPW):
                        nc.tensor.matmul(out=aggP[:],
                                         lhsT=oh[:, t * P:(t + 1) * P],
                                         rhs=msg[:, t * FEA:(t + 1) * FEA],
                                         start=(t == 0), stop=(t == TPW - 1))
                    pay = agg_own[:, w * FEA:(w + 1) * FEA]
                    nc.vector.tensor_copy(out=pay, in_=aggP[:])
                    sq = sp.tile([P, FEA], bf16, tag="sq")
                    nc.vector.tensor_tensor(out=sq[:], in0=pay, in1=pay,
                                            op=OP.mult)
                    nc.tensor.matmul(out=poolP[:, 0:FEA],
                                     lhsT=B_all[:, w * P:(w + 1) * P],
                                     rhs=pay, start=(w == 0),
                                     stop=(w == NW - 1),
                                     skip_group_check=True)
                    nc.tensor.matmul(out=poolP[:, FEA:P],
                                     lhsT=B_all[:, w * P:(w + 1) * P],
                                     rhs=sq[:], start=(w == 0),
                                     stop=(w == NW - 1),
                                     skip_group_check=True)

                # ---- stats + pool collective ----
                if not (DO_B and DO_ST):
                    continue
                poolp_sb = wp.tile([P, P], f32, tag="poolp_sb")
                nc.vector.tensor_copy(out=poolp_sb[:], in_=poolP[:])
                stP = paux.tile([1, P], f32, space="PSUM", tag="aux1")
                nc.tensor.matmul(out=stP[:], lhsT=onesf[:],
                                 rhs=poolp_sb[:], start=True, stop=True)
                str_sb = sp.tile([1, P], f32, tag="str_sb")
                nc.vector.tensor_copy(out=str_sb[:], in_=stP[:])
                nc.sync.dma_start(out=st_bounce[0:P, :], in_=poolp_sb[:])
                nc.sync.dma_start(out=st_bounce[P:P + 1, :], in_=str_sb[:])
                nc.gpsimd.collective_compute(
                    "AllGather", OP.bypass, replica_groups=rg,
                    ins=[st_bounce[:]], outs=[st_all[:]])

                # reduce the 8 cores' [P+1, P] blocks
                pool8 = wp.tile([P, P], f32, tag="pool8")
                nc.sync.dma_start(out=pool8[:], in_=st_all[0:P, :])
                st8 = sp.tile([NCORES, P], f32, tag="st8")
                nc.sync.dma_start(
                    out=st8[:],
                    in_=st_all[:].rearrange("(c q) s -> c q s", c=NCORES)
                    [:, P:P + 1, :])
                for c in range(1, NCORES):
                    pl = wp.tile([P, P], f32, tag="pl")
                    nc.sync.dma_start(
                        out=pl[:], in_=st_all[c * (P + 1):c * (P + 1) + P, :])
                    nc.vector.tensor_tensor(out=pool8[:], in0=pool8[:],
                                            in1=pl[:], op=OP.add)
                stspP = paux.tile([1, P], f32, space="PSUM", tag="aux1")
                nc.tensor.matmul(out=stspP[:], lhsT=onesf[0:NCORES, :],
                                 rhs=st8[:], start=True, stop=True)
                m1 = sp.tile([1, P], f32, tag="m1")
                nc.vector.tensor_scalar_mul(m1[:], stspP[:], 1.0 / N_NODES)
                # BN constants: sc = gam*rstd ; sh = bet - mu*sc
                mu2 = sp.tile([1, FEA], f32, tag="mu2")
                nc.vector.tensor_tensor(out=mu2[:], in0=m1[:, 0:FEA],
                                        in1=m1[:, 0:FEA], op=OP.mult)
                var = sp.tile([1, FEA], f32, tag="var")
                nc.vector.tensor_tensor(out=var[:], in0=m1[:, FEA:P],
                                        in1=mu2[:], op=OP.subtract)
                lnv = sp.tile([1, FEA], f32, tag="lnv")
                nc.scalar.activation(out=lnv[:], in_=var[:], func=AF.Ln,
                                     bias=eps_t[:])
                rstd = sp.tile([1, FEA], f32, tag="rstd")
                nc.scalar.activation(out=rstd[:], in_=lnv[:], func=AF.Exp,
                                     scale=-0.5)
                ssrow = sp.tile([1, P], f32, tag="ssrow")
                nc.vector.tensor_tensor(
                    out=ssrow[:, 0:FEA], in0=rstd[:],
                    in1=gam_sb[:, li * FEA:(li + 1) * FEA], op=OP.mult)
                msc = sp.tile([1, FEA], f32, tag="msc")
                nc.vector.tensor_tensor(out=msc[:], in0=m1[:, 0:FEA],
                                        in1=ssrow[:, 0:FEA], op=OP.mult)
                nc.vector.tensor_tensor(
                    out=ssrow[:, FEA:P],
                    in0=bet_sb[:, li * FEA:(li + 1) * FEA],
                    in1=msc[:], op=OP.subtract)
                colp = paux.tile([P, 1], f32, space="PSUM", tag="aux1")
                nc.tensor.transpose(out=colp[:], in_=ssrow[:],
                                    identity=identf[0:1, 0:1])
                col = sp.tile([P, 1], f32, tag="col")
                nc.vector.tensor_copy(out=col[:], in_=colp[:])
                Mp = paux.tile([P, P], f32, space="PSUM", tag="aux1")
                nc.tensor.transpose(out=Mp[:],
                                    in_=col[:].to_broadcast([P, P]),
                                    identity=identf[:])
                M = sp.tile([P, P], bf16, tag="M")
                nc.vector.tensor_copy(out=M[:], in_=Mp[:])
                Mf = sp.tile([P, P], f32, tag="Mf")
                nc.vector.tensor_copy(out=Mf[:], in_=Mp[:])

                # ---- D: wide BN apply + residual on own shard ----
                nc.vector.tensor_tensor(
                    out=bnt[:].rearrange("p (w f) -> p w f", w=NW),
                    in0=agg_own[:].rearrange("p (w f) -> p w f", w=NW),
                    in1=M[:, None, 0:FEA].to_broadcast([P, NW, FEA]),
                    op=OP.mult)
                nc.vector.tensor_tensor(
                    out=bnt[:].rearrange("p (w f) -> p w f", w=NW),
                    in0=bnt[:].rearrange("p (w f) -> p w f", w=NW),
                    in1=M[:, None, FEA:P].to_broadcast([P, NW, FEA]),
                    op=OP.add)
                nc.vector.tensor_tensor(out=h_own[:], in0=h_own[:],
                                        in1=bnt[:], op=OP.add)

                # ---- pool accumulation: poolacc += pool8*sc + cnt*sh ----
                t1 = sp.tile([P, FEA], f32, tag="t1")
                nc.vector.tensor_tensor(out=t1[:], in0=pool8[:, 0:FEA],
                                        in1=Mf[:, 0:FEA], op=OP.mult)
                nc.vector.tensor_tensor(out=poolacc[:], in0=poolacc[:],
                                        in1=t1[:], op=OP.add)
                t2 = sp.tile([P, FEA], f32, tag="t2")
                nc.vector.tensor_tensor(out=t2[:],
                                        in0=cnt_sb[:].to_broadcast([P, FEA]),
                                        in1=Mf[:, FEA:P], op=OP.mult)
                nc.vector.tensor_tensor(out=poolacc[:], in0=poolacc[:],
                                        in1=t2[:], op=OP.add)

            # ---------------- pooling + head ----------------
            nc.vector.tensor_tensor(out=poolacc[:], in0=poolacc[:],
                                    in1=hpool_sb[:], op=OP.add)
            cmx = sp.tile([P, 1], f32, tag="cmx")
            nc.vector.tensor_scalar_max(cmx[:], cnt_sb[:], 1.0)
            rec = sp.tile([P, 1], f32, tag="rec")
            nc.vector.reciprocal(rec[:], cmx[:])
            pooled = wp.tile([P, FEA], bf16, tag="pooled")
            nc.vector.tensor_scalar_mul(pooled[:], poolacc[:], rec[:])

            def head_mm(in_bf, k, n, W_list, bmat, act):
                outp = pmm.tile([P, LATENT * CH], f32, space="PSUM",
                                tag="mm4")
                nchunks = (k + P - 1) // P
                for i in range(nchunks):
                    kk = min(P, k - i * P)
                    tp = ptp.tile([P, P], bf16, space="PSUM", tag="tp")
                    nc.tensor.transpose(out=tp[0:kk, :],
                                        in_=in_bf[:, i * P:i * P + kk],
                                        identity=ident[:])
                    tT = wp.tile([P, P], bf16, tag="tT")
                    nc.vector.tensor_copy(out=tT[0:kk, :], in_=tp[0:kk, :])
                    nc.tensor.matmul(out=outp[:, 0:n], lhsT=tT[0:kk, :],
                                     rhs=W_list[i][0:kk, 0:n],
                                     start=(i == 0), stop=(i == nchunks - 1))
                zb = wp.tile([P, n], f32, tag="zb")
                nc.vector.tensor_tensor(out=zb[:], in0=outp[:, 0:n],
                                        in1=bmat[:, 0:n], op=OP.add)
                if not act:
                    return zb
                ez = wp.tile([P, n], bf16, tag="ez")
                nc.scalar.activation(out=ez[:], in_=zb[:], func=AF.Exp)
                g = wp.tile([P, n], bf16, tag="g")
                nc.scalar.activation(out=g[:], in_=ez[:], func=AF.Ln,
                                     bias=1.0)
                return g

            g1 = head_mm(pooled, FEA, HID, [W1_sb], b1_sb, True)
            g2 = head_mm(g1, HID, HID, [W2a_sb, W2b_sb], b2_sb, True)
            dosv = head_mm(g2, HID, LATENT * CH, [W3a_sb, W3b_sb], b3_sb,
                           False)
            nc.sync.dma_start(out=dos_out[:], in_=dosv[:])

            hd = wp.tile([P, FEA], f32, tag="hdbg")
            nc.vector.tensor_copy(out=hd[:], in_=h_own[:, 0:FEA])
            nc.sync.dma_start(out=hdbg_out[:], in_=hd[:])

    nc.compile()
    return nc


# ======================== host side ========================

def fast_bf16(a):
    a = np.ascontiguousarray(a, np.float32)
    u = a.view(np.uint32)
    r = ((u >> 16) & np.uint32(1)) + np.uint32(0x7FFF)
    return ((u + r) >> 16).astype(np.uint16).view(ml_dtypes.bfloat16)


def _prepare(x, edge_index, edge_attr, batch, emb_W, emb_b):
    ei0 = np.asarray(edge_index[0])
    ei1 = np.asarray(edge_index[1])
    deg = np.bincount(ei1.astype(np.intp, copy=False), minlength=N_NODES)
    nbins = NCORES * NW
    order = np.argsort(-deg, kind="stable")
    nrounds = (N_NODES + nbins - 1) // nbins
    pad = nrounds * nbins - N_NODES
    padded = np.concatenate([order, np.full(pad, -1, np.int64)])
    grid = padded.reshape(nrounds, nbins)
    grid[1::2] = grid[1::2, ::-1]
    flat = grid.ravel()
    v = flat >= 0
    binidx = np.tile(np.arange(nbins, dtype=np.int64), nrounds)
    rndidx = np.repeat(np.arange(nrounds, dtype=np.int64), nbins)
    bin_of = np.empty(N_NODES, np.int64)
    slot_of = np.empty(N_NODES, np.int64)
    bin_of[flat[v]] = binidx[v]
    slot_of[flat[v]] = rndidx[v]
    newid = (bin_of // NW) * VC + (bin_of % NW) * P + slot_of

    src_new = newid[ei0].astype(np.int32)
    dst_new = newid[ei1].astype(np.int32)
    gwin = (dst_new >> 7).astype(np.int32)
    eorder = np.argsort(gwin, kind="stable")
    gwin_s = gwin[eorder]
    counts = np.bincount(gwin_s, minlength=nbins)
    assert counts.max() <= WID, f"window overflow {counts.max()}"
    offs = np.concatenate([[0], np.cumsum(counts)])
    within = np.arange(len(gwin_s), dtype=np.int64) - offs[gwin_s]
    tgt = (gwin_s // NW).astype(np.int64) * EC + \
        (gwin_s % NW).astype(np.int64) * WID + within

    E = len(ei0)
    gidx = np.full(NCORES * EC, E, np.int64)
    gidx[tgt] = eorder

    src_ext = np.empty(E + 1, np.int32)
    src_ext[:E] = src_new
    src_ext[E] = 0
    dstloc_ext = np.empty(E + 1, np.int32)
    dstloc_ext[:E] = dst_new % VC
    dstloc_ext[E] = 0
    slot_ext = np.empty(E + 1, np.float32)
    slot_ext[:E] = (dst_new & 127).astype(np.float32)
    slot_ext[E] = -1.0

    # edge slot (w, p, t); resident layout [P, NW*TPW] col w*16+t
    def res_layout(a):
        return np.ascontiguousarray(
            a.reshape(NCORES, NW, P, TPW).transpose(0, 2, 1, 3)
            .reshape(NCORES, P, NT))

    src_res = res_layout(src_ext[gidx])
    dst_res = res_layout(dstloc_ext[gidx])
    slot_res = res_layout(slot_ext[gidx]).astype(ml_dtypes.bfloat16)

    # eaT [41, EC] with cols ordered (w, t, p)
    ea_bf = np.empty((E + 1, EDGE_DIM), ml_dtypes.bfloat16)
    ea_bf[:E] = fast_bf16(np.asarray(edge_attr))
    ea_bf[E] = 0
    gidx_ct = gidx.reshape(NCORES, NW, P, TPW).transpose(0, 1, 3, 2) \
        .reshape(NCORES, EC)
    ea_cols = ea_bf[gidx_ct]                      # [NCORES, EC, 41]
    eaT = np.ascontiguousarray(ea_cols.transpose(0, 2, 1))

    # h0 on host (f32 matmul), permuted into shard rows
    h0 = np.asarray(x, np.float32) @ np.asarray(emb_W, np.float32) + \
        np.asarray(emb_b, np.float32)
    old_of_new = np.full(VTOT, -1, np.int64)
    old_of_new[newid] = np.arange(N_NODES)
    sel = old_of_new.copy()
    sel[sel < 0] = N_NODES
    h0_ext = np.empty((N_NODES + 1, FEA), np.float32)
    h0_ext[:N_NODES] = h0
    h0_ext[N_NODES] = 0.0
    h0_sh = fast_bf16(h0_ext[sel]).reshape(NCORES, VC, FEA)

    # own nodes' graph ids [P, NW] per core (pad -> 500)
    bs = np.asarray(batch).astype(np.float32)
    bs_ext = np.concatenate([bs, [500.0]])
    bo = bs_ext[sel].reshape(NCORES, NW, P).transpose(0, 2, 1)
    batcho = np.ascontiguousarray(bo).astype(ml_dtypes.bfloat16)

    # host-side pooling of h0 + counts (batch sorted in original ids)
    bi = np.asarray(batch).astype(np.int64)
    starts = np.searchsorted(bi, np.arange(NUM_GRAPHS))
    hpool = np.add.reduceat(h0, starts, axis=0).astype(np.float32)
    ends = np.concatenate([starts[1:], [N_NODES]])
    hpool[starts == ends] = 0.0
    cnt = np.bincount(bi, minlength=NUM_GRAPHS).astype(np.float32)

    return dict(src=src_res, dst=dst_res, slot=slot_res, eaT=eaT,
                h0=h0_sh, batcho=batcho, hpool=hpool,
                cnt=cnt.reshape(NUM_GRAPHS, 1), newid=newid)


def _prep_weights(ins):
    def neg_f(w):
        w = np.array(w, np.float32)
        w[..., :FEA] = -w[..., :FEA]
        return w

    Wi_s = np.zeros((FEA, N_CONV * 128), np.float32)
    Wj_s = np.zeros((FEA, N_CONV * 128), np.float32)
    # duplicated across both partition halves at the end of this fn
    Wea_s = np.zeros((EDGE_DIM, PC5), np.float32)
    bias_s = np.zeros((P, N_CONV * 128), np.float32)
    gam_s = np.zeros((1, N_CONV * FEA), np.float32)
    bet_s = np.zeros((1, N_CONV * FEA), np.float32)
    for i in range(N_CONV):
        Wf = np.asarray(ins["lin_f_W"][i])
        Ws = np.asarray(ins["lin_s_W"][i])
        bfv = np.asarray(ins["lin_f_b"][i])
        bsv = np.asarray(ins["lin_s_b"][i])
        Wi_s[:, i * 128:i * 128 + 128] = neg_f(
            np.concatenate([Wf[0:64], Ws[0:64]], axis=1))
        Wj_s[:, i * 128:i * 128 + 128] = neg_f(
            np.concatenate([Wf[64:128], Ws[64:128]], axis=1))
        Wea_s[:, i * 128:i * 128 + 128] = neg_f(
            np.concatenate([Wf[128:169], Ws[128:169]], axis=1))
        bias_s[:, i * 128:i * 128 + 128] = np.broadcast_to(
            neg_f(np.concatenate([bfv, bsv])), (P, 128))
        gam_s[0, i * FEA:(i + 1) * FEA] = np.asarray(ins["bn_gamma"][i])
        bet_s[0, i * FEA:(i + 1) * FEA] = np.asarray(ins["bn_beta"][i])
    Wi_s = np.concatenate([Wi_s, Wi_s], axis=0)
    Wj_s = np.concatenate([Wj_s, Wj_s], axis=0)
    return Wi_s, Wj_s, Wea_s, bias_s, gam_s, bet_s


def kernel(**inputs):
    x = np.asarray(inputs["x"], np.float32)
    edge_index = np.asarray(inputs["edge_index"])
    edge_attr = np.asarray(inputs["edge_attr"], np.float32)
    batch = np.asarray(inputs["batch"])

    prep = _prepare(x, edge_index, edge_attr, batch,
                    inputs["emb_W"], inputs["emb_b"])
    Wi_s, Wj_s, Wea_s, bias_s, gam_s, bet_s = _prep_weights(inputs)

    bf = ml_dtypes.bfloat16
    iota = np.broadcast_to(np.arange(P, dtype=np.float32), (P, P)).astype(bf)
    b1 = np.broadcast_to(np.asarray(inputs["head_b1"], np.float32), (P, HID))
    b2 = np.broadcast_to(np.asarray(inputs["head_b2"], np.float32), (P, HID))
    b3 = np.broadcast_to(np.asarray(inputs["head_b3"], np.float32),
                         (P, LATENT * CH))
    hpool = np.zeros((P, FEA), np.float32)
    hpool[:NUM_GRAPHS] = prep["hpool"]
    cnt = np.zeros((P, 1), np.float32)
    cnt[:NUM_GRAPHS] = prep["cnt"]

    if "nc" not in _cache:
        _cache["nc"] = _build()
    nc = _cache["nc"]

    in_maps = []
    for c in range(NCORES):
        in_maps.append(dict(
            h0_in=np.ascontiguousarray(prep["h0"][c]),
            srcr_in=prep["src"][c], dstr_in=prep["dst"][c],
            slotr_in=prep["slot"][c], eaT_in=prep["eaT"][c],
            batcho_in=prep["batcho"][c],
            iota_in=np.asarray(iota),
            Wi_in=Wi_s.astype(bf), Wj_in=Wj_s.astype(bf),
            Wea_in=Wea_s.astype(bf), bias_in=bias_s.astype(bf),
            gam_in=gam_s, bet_in=bet_s,
            hpool_in=hpool, cnt_in=cnt,
            W1_in=np.asarray(inputs["head_W1"], np.float32).astype(bf),
            W2_in=np.asarray(inputs["head_W2"], np.float32).astype(bf),
            W3_in=np.asarray(inputs["head_W3"], np.float32).astype(bf),
            b1_in=np.ascontiguousarray(b1), b2_in=np.ascontiguousarray(b2),
            b3_in=np.ascontiguousarray(b3),
        ))
    res = run_bass_kernel_spmd(nc, in_maps, list(range(NCORES)))
    dos = res.results[0]["dos_out"].astype(np.float32)
    _cache["hdbg"] = res.results[0]["hdbg_out"]
    _cache["newid"] = prep["newid"]
    return dos[:NUM_GRAPHS].reshape(NUM_GRAPHS, CH, LATENT)
